# revision 1
# baseline (speedup 1.0000x reference)
"""Trainium2 Bass kernel for nn_AttnLayer (additive attention over history).

Math (per batch b, S = T*N = 8192 positions, A = H = 128):
    c[b]      = cur_h[b] @ Wx_w.T + Wx_b + Wh_b                  (host, tiny)
    pj[a,s]   = alpha * (sum_h Wh[a,h] hist[s,h] + c[b,a])       (PE, [a,s] layout)
    tnh[a,s]  = tanh(pj/alpha)   ACT share: native tanh (scale arg is free)
                                 DVE share: custom fused 7-stage op = clamped
                                 odd deg-5 poly z(TK0+q(TK1+q)), q=z^2,
                                 z = clamp(pj, +-BZ)   (|err| <= 1.7e-2)
    score[s]  = sum_a v[a] tnh[a,s]    (PE matvec: tnh chunk stationary, v moving,
                                        out free size 1 -> ~free)
    esc       = exp(score)  (bf16)     (ACT; |score| <= ||v||_1 ~ 5.6, exp safe)
    attn_h[h] = (sum_s esc[s] hist[s,h]) / sum_s esc[s]
                (PE pass-2: histN tile stationary, esc column moving -> out
                 [128,1] at ~1 cycle/tile; sum(esc) via a ones stationary)
    out[b]    = cur_h[b] + attn_h                                (host, tiny)

Layouts (host pre-packed, all history fp8 e4m3):
    histT8[b][p][j*8192+s] = hist[b,s,64j+p], row 64 = ones   pass-1 moving
        (DoubleRow: contraction 64 partitions x 2; the ones row pairs with a
         fp8 coarse+residual alpha*c bias pair in the stationary row 64, so
         the bias costs zero PE cycles)
    histN8[b][p][i*128+h]  = hist[b,128i+p,h]                 pass-2 stationary

Pipeline: 64 chunks of 512 tanh columns, batch b's tail chunks interleaved
1:1 with batch b+1's head chunks (no inter-batch pipeline boundary); tanh
alternates ACT/DVE per chunk; matvec/exp (one [128,64] instruction per
batch)/pass-2/finish tasks are emitted behind the producer front with tuned
lags so the in-order PE queue never head-blocks on an unmet dependency.
PSUM: 4 pj banks + 2 score banks + 2 acc/z banks. The kernel is DMA-bound
(2 fp8 copies of history, 8.4 MB/core, ~24.1 us of serialized transfers)
with ACT (~21.9 us) and DVE (~22.4 us) fitting inside the stream.

Sharding: data-parallel over batch B=32 across 8 cores (4 batches/core).
"""

import os
import sys
from contextlib import ExitStack

import numpy as np
import ml_dtypes

for _p in (
    "/root/.axon_site",
    "/root/.axon_site/_ro/trn_rl_repo",
    "/root/.axon_site/_ro/pypackages",
    "/opt/trn_rl_repo",
):
    if os.path.isdir(_p) and _p not in sys.path:
        sys.path.append(_p)

import concourse.bass as bass  # noqa: E402
import concourse.tile as tile  # noqa: E402
from concourse import bacc, mybir  # noqa: E402
import concourse.bass_utils as bass_utils  # noqa: E402
import concourse.dve_ops as dve_ops  # noqa: E402
from concourse.dve_spec import (  # noqa: E402
    Spec, Src0, Src1, C0, C1, C2, maxx, minn, lower, _has_src1,
)
from concourse.dve_uop import DveOpSpec  # noqa: E402
from concourse.dve_table_gen import dve_ver_for  # noqa: E402

BF16 = mybir.dt.bfloat16
FP8 = mybir.dt.float8e4
F32 = mybir.dt.float32
NPBF16 = ml_dtypes.bfloat16
NPFP8 = ml_dtypes.float8_e4m3

B, T, N, HID, ATTN = 32, 64, 128, 128, 128
NCORES = 8
BL = B // NCORES          # batches per core
S = T * N                 # history positions per batch (8192)
P = 128
HP = 64                   # half partitions (DoubleRow contraction = 64 x 2)
KC = 512                  # chunk columns (1 psum bank; tanh instruction size)
NKC = S // KC             # chunks per batch (16)
NPC = int(os.environ.get("K_NPC", "2"))  # histT DMA pieces per batch
PJB = int(os.environ.get("K_PJB", "4"))   # pj psum buffers (banks)
# engine plan per batch: per chunk, 'A' = ACT tanh, 'D' = DVE poly tanh
# engine per SEQUENCE position; first pair swapped (DVE starts chunk 0
# while ACT's queue is still draining the startup weight DMAs)
PLAN = os.environ.get("K_PLAN", "DA" + "AD" * 31)
LAG = int(os.environ.get("K_LAG", "4"))
P2G = int(os.environ.get("K_P2G", "2"))   # pass-2 emission groups/batch
P2SPREAD = int(os.environ.get("K_P2SPREAD", "1"))
P2DELAY_A = int(os.environ.get("K_P2DELAY_A", "1"))
P2DELAY_B = int(os.environ.get("K_P2DELAY_B", "2"))
FIN_DELAY = int(os.environ.get("K_FIN_DELAY", "12"))
MVQ = int(os.environ.get("K_MVQ", "1"))   # max mv pops per producer step
EXPD = int(os.environ.get("K_EXPD", "0"))  # extra exp delay past its last mv
NT = S // P               # pass-2 s-tiles per batch (64)
HN = NT * P               # histN8 bytes/partition ([128, 8192])

# clamped odd deg-5 tanh fit (z = ALPHA*x clamped to +-BZ):
# tanh(x) ~= z*(TK0 + q*(TK1 + q)), q = z*z;  max abs err 1.61e-2
ALPHA = 0.447118
TK0 = 2.107214
TK1 = -2.107472
BZ = 0.983659

_cache = {}
TASKLOG = []   # (n_instructions_at_mark, label) for trace attribution


def _mark(tc, label):
    try:
        TASKLOG.append((int(tc.nc.next_id()), label))
    except Exception:
        pass


def _register_tanh5():
    """Register the fused clamp+poly tanh DVE op (7 ALU stages, 1 uop)."""
    name = "TANH5_CLAMP_ANT"
    for op in dve_ops.OPS:
        if op.name == name:
            return op
    z = minn(maxx(Src0, C0), C1)
    q = z * z
    body = ((q + C2) * q + Src1) * z

    def ref(in0, in1, c0, c1, c2):
        zz = np.minimum(np.maximum(in0.astype(np.float32), c0), c1)
        qq = zz * zz
        return ((qq + c2) * qq + in1) * zz

    spec = Spec(body=body, reference=ref)
    ver = dve_ver_for("TRN2")
    free = [r for r in range(1, 32) if r not in dve_ops._SUB_OPCODE_FOR_NAME.values()]
    row = free[0]
    s = DveOpSpec(name=name, opcode=row, uops=lower(spec, ver=ver),
                  rd1_en=_has_src1(spec))
    op = dve_ops.DveOp(name, spec, subdim=False, uops_sha={ver: s.sha(ver)})
    dve_ops.OPS.append(op)
    dve_ops._SUB_OPCODE_FOR_NAME[name] = row
    dve_ops.CUSTOM_DVE_SPECS[name] = spec
    return op


TANH5 = _register_tanh5()


def _build_kernel(tc, histT8, histN8, wpack8, v16, acc_out):
    nc = tc.nc
    AF = mybir.ActivationFunctionType
    DR = mybir.MatmulPerfMode.DoubleRow
    with ExitStack() as ctx:
        wpool = ctx.enter_context(tc.tile_pool(name="w", bufs=1))
        bigT = ctx.enter_context(tc.tile_pool(name="bigT", bufs=BL * NPC + 4))
        bigN = ctx.enter_context(tc.tile_pool(name="bigN", bufs=2 * BL))
        pjp = ctx.enter_context(tc.tile_pool(name="pj", bufs=PJB, space="PSUM"))
        sap = ctx.enter_context(tc.tile_pool(name="sa", bufs=int(os.environ.get("K_SAB", "2")), space="PSUM"))
        azp = ctx.enter_context(tc.tile_pool(name="az", bufs=int(os.environ.get("K_AZB", "2")), space="PSUM"))
        tnhp = ctx.enter_context(tc.tile_pool(name="tnh", bufs=int(os.environ.get("K_TNB", "10"))))
        escp = ctx.enter_context(tc.tile_pool(name="esc", bufs=3))
        accsb = ctx.enter_context(tc.tile_pool(name="accsb", bufs=2))

        # --- small weights first (ACT ring, one combined DMA: doesn't
        # block SP load issue, barely delays first tanh) ---
        # per-batch stationary [65, 2, 128]: rows 0-63 = whT8 (alpha*Wh),
        # row 64 = (j0: fp8-coarse alpha*c[b], j1: residual) bias pair; the
        # moving histT8 carries a matching all-ones row 64 -> bias lands in
        # pj via the same DoubleRow matmul, costing zero extra PE cycles.
        w8 = wpool.tile([HP + 1, BL * 2 * P], FP8, tag="w8")
        _wring = nc.scalar if os.environ.get("K_WSYNC", "0") == "0" else nc.sync
        _wring.dma_start(
            w8[:].rearrange("p (b m) -> p b m", b=BL),
            wpack8.rearrange("b p m -> p b m"),
        )
        wbs = [
            w8[:, 2 * P * b : 2 * P * (b + 1)].rearrange("p (two m) -> p two m", two=2)
            for b in range(BL)
        ]
        vsb = wpool.tile([P, 1], BF16, tag="v16")
        _wring.dma_start(vsb[:], v16)
        k0t = wpool.tile([P, KC], F32, tag="k0")
        nc.gpsimd.memset(k0t[:], TK0)
        ones1t = wpool.tile([P, 1], BF16, tag="ones1t")
        nc.gpsimd.memset(ones1t[:], 1.0)

        # --- history loads (sync ring), interleaved so histN8[b] lands just
        # before batch b's tail needs it ---
        Tbs = {}
        Nbs = {}

        # piece column-lists per batch; optional small leading piece on
        # batch 0 so the first chunk's data lands early
        lead = int(os.environ.get("K_LEAD", "0"))
        p0 = [S // NPC] * NPC
        if lead:
            p0 = [lead, S // NPC - lead] + [S // NPC] * (NPC - 1)
        PIECES = [p0] + [[S // NPC] * NPC] * (BL - 1)

        def load_T(b):
            _mark(tc, f"loadT({b})")
            Tbs.setdefault(b, [])
            q = len(Tbs[b])
            ps = PIECES[b][q]
            off = sum(PIECES[b][:q])
            t = bigT.tile([HP + 1, 2 * ps], FP8, tag="histT",
                          name=f"histT{b}_{q}")
            src = histT8[b].rearrange("p (two s) -> p two s", two=2)
            nc.sync.dma_start(
                t[:].rearrange("p (two s) -> p two s", two=2),
                src[:, :, off : off + ps],
            )
            Tbs[b].append((t, ps))

        def load_N(b):
            _mark(tc, f"loadN({b})")
            t = bigN.tile([P, HN // 2], FP8, tag="histN")
            Nbs.setdefault(b, [])
            q = len(Nbs[b])
            nc.sync.dma_start(t[:], histN8[b][:, HN // 2 * q : HN // 2 * (q + 1)])
            Nbs[b].append(t)

        # --- interleaved chunk sequence: batch b's tail chunks (8..15)
        # alternate 1:1 with batch b+1's head chunks (0..7), so a batch's
        # mv/exp/pass-2 tail always overlaps an already-flowing stream of
        # fresh chunks -- there are no pipeline boundaries between batches.
        chunk_seq = [(0, kc) for kc in range(NKC // 2)]
        for b in range(BL):
            tail = [(b, kc) for kc in range(NKC // 2, NKC)]
            head = [(b + 1, kc) for kc in range(NKC // 2)] if b + 1 < BL else []
            for i in range(NKC // 2):
                chunk_seq.append(tail[i])
                if head:
                    chunk_seq.append(head[i])
        pos_of = {bk: i for i, bk in enumerate(chunk_seq)}

        # loads sorted by first-use position in the chunk sequence
        loads = []
        for b in range(BL):
            cum = 0
            for q in range(len(PIECES[b])):
                loads.append((pos_of[(b, cum // KC)], "T", b))
                cum += PIECES[b][q]
            loads.append((pos_of[(b, NKC // 2 - 1)] + LAG + P2DELAY_A, "N", b))
            loads.append((pos_of[(b, NKC - 1)] + LAG + P2DELAY_B, "N", b))
        for _, kind, b in sorted(loads, key=lambda t: t[0]):
            (load_T if kind == "T" else load_N)(b)

        def histT_slice(b, s0, ncols):
            """[64, 2, ncols] moving slice for s-range [s0, s0+ncols)."""
            for t, piece_s in Tbs[b]:
                if s0 < piece_s:
                    ap = t[:].rearrange("p (two s) -> p two s", two=2)
                    return ap[:, :, s0 : s0 + ncols]
                s0 -= piece_s
            raise AssertionError("bad slice")

        scoreaccs = {}
        accz = {}
        tnhs = {}

        def prod(b, kc, eng):
            """pass-1 chunk: one DoubleRow matmul (bias in row 64) + tanh."""
            _mark(tc, f"prod{eng}({b},{kc})")
            pj = pjp.tile([P, KC], F32, tag="pj")
            nc.tensor.matmul(
                pj[:],
                wbs[b],
                histT_slice(b, KC * kc, KC),
                start=True, stop=True,
                perf_mode=DR,
            )
            tnh = tnhp.tile([P, KC], BF16, tag="tnh")
            if eng == "D":
                nc.vector._custom_dve(
                    TANH5, out=tnh[:], in0=pj[:], in1=k0t[:],
                    s0=-BZ, s1=BZ, imm2=TK1,
                )
            else:
                nc.scalar.activation(
                    tnh[:], pj[:], AF.Tanh, scale=1.0 / ALPHA,
                )
            tnhs[(b, kc)] = tnh

        def matvecs(b, kc):
            """score columns for chunk kc: 4 matvecs, out [128,1] each.
            sa layout (one psum bank): cols 0..63 score (tile-major),
            col 64 acc_h, cols 96..127 z partials."""
            if kc == 0:
                scoreaccs[b] = sap.tile([P, NT], F32, tag="sa", name=f"sa{b}")
                accz[b] = azp.tile([P, 72], F32, tag="az", name=f"az{b}")
            sa = scoreaccs[b]
            tnh = tnhs.pop((b, kc))
            _mark(tc, f"mv({b},{kc})")
            for m in range(KC // P):
                i = (KC // P) * kc + m           # s-tile index
                nc.tensor.matmul(
                    sa[:, i : i + 1],
                    tnh[:, P * m : P * (m + 1)],
                    vsb[:],
                    start=True, stop=True,
                )

        escs = {}

        def exp_task(b, half):
            """exp of the whole batch's score tiles -> bf16 esc (one instr)."""
            sa = scoreaccs[b]
            _mark(tc, f"exp({b})")
            esc = escp.tile([P, NT], BF16, tag="esc", name=f"esc{b}")
            escs[b] = esc
            nc.scalar.activation(esc[:], sa[:, 0:NT], AF.Exp)
            scoreaccs.pop(b)

        def p2_task(b, g):
            """pass-2: esc bf16 is the MOVING operand (out free size 1 ->
            ~free on PE); histN [128s, 128h] tiles are the stationary.
            acc_h partials in az cols 0..7; z partials at cols 8..71."""
            _mark(tc, f"p2({b},{g})")
            az = accz[b]
            esc = escs[b]
            acc = az[:, g : g + 1]             # per-group partial (own group)
            gn = NT // P2G
            for i in range(gn * g, gn * (g + 1)):
                nb = Nbs[b][i // (NT // 2)]
                nc.tensor.matmul(
                    acc,
                    nb[:, P * (i % (NT // 2)) : P * (i % (NT // 2) + 1)],
                    esc[:, i : i + 1],
                    start=(i == gn * g), stop=(i == gn * (g + 1) - 1),
                )
            if g % (P2G // 2) == P2G // 2 - 1:
                half = g // (P2G // 2)
                hs = NT // 2
                nc.tensor.matmul(
                    az[0:1, 8 + hs * half : 8 + hs * (half + 1)],
                    ones1t[:],
                    esc[:, hs * half : hs * (half + 1)],
                    start=True, stop=True,
                )

        def fin_task(b):
            """acc psum -> sbuf copy + output DMA (late, never head-blocks)."""
            _mark(tc, f"fin({b})")
            az = accz.pop(b)
            escs.pop(b)
            ob = accsb.tile([P, 2], F32, tag="ob")
            nc.vector.memset(ob[:, 1:2], 0.0)
            nc.vector.tensor_reduce(
                ob[:, 0:1], az[:, 0:P2G],
                axis=mybir.AxisListType.X, op=mybir.AluOpType.add,
            )
            nc.vector.tensor_reduce(
                ob[0:1, 1:2], az[0:1, 8 : 8 + NT],
                axis=mybir.AxisListType.X, op=mybir.AluOpType.add,
            )
            _oring = nc.gpsimd if os.environ.get("K_OPOOL", "0") == "1" else nc.sync
            _oring.dma_start(acc_out[b], ob[:])

        # --- software pipeline: producers in interleaved order; consumers
        # pop when ready (lagged), earliest-ready first ---
        import heapq

        pend = []
        prod_idx = 0
        seq = 0

        def push(ready, task):
            nonlocal seq
            heapq.heappush(pend, (ready, seq, task))
            seq += 1

        def emit(t):
            if t[0] == "mv":
                matvecs(t[1], t[2])
            elif t[0] == "exp":
                exp_task(t[1], t[2])
            elif t[0] == "fin":
                fin_task(t[1])
            else:
                p2_task(t[1], t[2])

        for i, (b, kc) in enumerate(chunk_seq):
            prod(b, kc, PLAN[i % len(PLAN)])
            prod_idx += 1
            push(prod_idx + LAG, ("mv", b, kc))
            if kc == NKC - 1:
                push(prod_idx + LAG + EXPD, ("exp", b, 1))
                for g in range(P2G):
                    push(prod_idx + LAG + P2DELAY_B + g * P2SPREAD, ("p2", b, g))
                fd = max(FIN_DELAY,
                         LAG + P2DELAY_B + (P2G - 1) * P2SPREAD + 1)
                push(prod_idx + fd, ("fin", b))
            # strict pacing: at most one mv + one tail task per step, so
            # PE keeps a main directly before every lagged consumer
            ready = []
            while pend and pend[0][0] <= prod_idx:
                ready.append(heapq.heappop(pend))
            n_mv = 0
            n_tail = 0
            for item in ready:
                kind = item[2][0]
                if kind == "mv" and n_mv < MVQ:
                    n_mv += 1
                    emit(item[2])
                elif kind != "mv" and n_tail == 0:
                    n_tail = 1
                    emit(item[2])
                else:
                    heapq.heappush(pend, item)
        while pend:
            emit(heapq.heappop(pend)[2])


def build():
    if "nc" in _cache:
        return _cache["nc"]
    nc = bacc.Bacc(
        "TRN2",
        target_bir_lowering=False,
        debug=False,
        enable_asserts=True,
        num_devices=NCORES,
    )
    histT8 = nc.dram_tensor("histT8", [BL, HP + 1, 2 * S], FP8, kind="ExternalInput").ap()
    histN8 = nc.dram_tensor("histN8", [BL, P, HN], FP8, kind="ExternalInput").ap()
    wpack8 = nc.dram_tensor("wpack8", [BL, HP + 1, 2 * P], FP8, kind="ExternalInput").ap()
    v16 = nc.dram_tensor("v16", [P, 1], BF16, kind="ExternalInput").ap()
    acc_out = nc.dram_tensor("acc_out", [BL, P, 2], F32, kind="ExternalOutput").ap()

    with tile.TileContext(nc) as tc:
        _build_kernel(tc, histT8, histN8, wpack8, v16, acc_out)
    nc.compile()
    _cache["nc"] = nc
    return nc


def make_in_maps(cur_h, history_h, Wx_w, Wx_b, Wh_w, Wh_b, v_w):
    """Host-side prep: shard over batch, pre-pack fp8 layouts, fold tiny ops."""
    cur_h = np.asarray(cur_h, np.float32)
    hist = np.asarray(history_h, np.float32).reshape(B, S, HID)
    c = (cur_h @ np.asarray(Wx_w, np.float32).T
         + np.asarray(Wx_b, np.float32)
         + np.asarray(Wh_b, np.float32))                      # [B, A]

    # pass-1 moving: histT8[b, p, j*S + s] = hist[b, s, 64j+p]; row 64 = ones
    hT = np.ascontiguousarray(hist.transpose(0, 2, 1))        # [B, H, S]
    histT8 = np.ones((B, HP + 1, 2 * S), NPFP8)
    histT8[:, :HP] = (hT.reshape(B, 2, HP, S).transpose(0, 2, 1, 3)
                      .reshape(B, HP, 2 * S).astype(NPFP8))
    # pass-2 stationary: histN8[b, p, i*128 + h] = hist[b, 128i+p, h]
    histN8 = np.ascontiguousarray(
        hist.reshape(B, NT, P, HID).transpose(0, 2, 1, 3)
    ).reshape(B, P, HN).astype(NPFP8)

    whT = np.asarray(Wh_w, np.float32).T * ALPHA              # [h, a] scaled
    whT8 = (whT.reshape(2, HP, ATTN).transpose(1, 0, 2)
            .reshape(HP, 2 * ATTN)).astype(NPFP8)             # [64, 2*128]

    v16 = np.ascontiguousarray(np.asarray(v_w, np.float32)[:, None]).astype(NPBF16)

    in_maps = []
    for q in range(NCORES):
        bsl = slice(BL * q, BL * (q + 1))
        cq = c[bsl] * ALPHA                                   # [BL, A]
        wpack8 = np.zeros((BL, HP + 1, 2 * ATTN), NPFP8)
        wpack8[:, :HP] = whT8[None]
        coarse = cq.astype(NPFP8)                             # bias row, j=0
        resid = (cq - coarse.astype(np.float32)).astype(NPFP8)  # j=1
        wpack8[:, HP, :ATTN] = coarse
        wpack8[:, HP, ATTN:] = resid
        in_maps.append(
            {
                "histT8": np.ascontiguousarray(histT8[bsl]),
                "histN8": np.ascontiguousarray(histN8[bsl]),
                "wpack8": wpack8,
                "v16": v16,
            }
        )
    return in_maps, cur_h


def finish_host(results, cur):
    outs = []
    for q in range(NCORES):
        acc = results[q]["acc_out"]          # [BL, P, 2]: attn_num | z
        outs.append(acc[:, :, 0] / acc[:, 0:1, 1])
    attn = np.concatenate(outs, axis=0)
    return (cur + attn).astype(np.float32)


def kernel(cur_h, history_h, Wx_w, Wx_b, Wh_w, Wh_b, v_w):
    nc = build()
    in_maps, cur = make_in_maps(cur_h, history_h, Wx_w, Wx_b, Wh_w, Wh_b, v_w)
    res = bass_utils.run_bass_kernel_spmd(nc, in_maps, core_ids=list(range(NCORES)))
    return finish_host(res.results, cur)


if __name__ == "__main__":
    build()
    print("build ok")

